# revision 1
# baseline (speedup 1.0000x reference)
"""ConvLSTM (T=16, B=4, C=32, HID=64, 64x64, 3x3 convs) on 8 Trainium2 cores.

Decomposition: 8 cores = batch(4) x H-halves(2). Each core owns 32 output rows
and recomputes a shrinking halo (rows 0..47-t at step t) so NO inter-core
communication is needed. The bottom-half cores get vertically flipped inputs
(and dy-flipped conv weights) so all 8 cores run the identical SPMD program.

Conv-as-matmul with dense tap packing: the 9 taps x 32ch (x2h) and 9 taps x
64ch (h2h) of the two 3x3 convs are packed into the 128-partition contraction
dim as shifted copies of the padded image, reaching the theoretical floor of
7 matmuls per 128-out-channel half per 512-pixel chunk:
    X1 = x taps (0,0)(0,1)(0,2)(1,0)   [4x32 rows, shifts 0,1,2,66]
    X2 = x taps (1,1)(1,2)(2,0)(2,1)   [shifts 67,68,132,133]
    HA @ dy*66, dy=0,1,2 = h taps (dy,0)(dy,1)   [2x64 rows, shifts 0,1]
    HB2 = h taps (0,2)(2,2)            [shifts 2,134]
    CMB = x tap (2,2) + h tap (1,2)    [96 rows, shifts 134 / 68]
x blocks are DMA'd from DRAM at shifted offsets (free); h blocks are written
by 5 small SBUF DMAs per chunk. All matmuls accumulate into one PSUM bank at
base partition 0 (mixed-base accumulation hangs the HW). MM_DT selects matmul
precision: bf16 (fast) or float32r (~20x more precise, ~1.7x slower PE).
"""
import sys
import os

for _p in ("/opt/trn_rl_repo", "/root/.axon_site"):
    if _p not in sys.path and os.path.isdir(_p):
        sys.path.append(_p)

import numpy as np

T, B, C, H, W, HID = 16, 4, 32, 64, 64, 64
HP, WP = 49, 66          # padded per-core image: 48 data rows + 1 top pad, 64+2 cols
FLAT = HP * WP           # 3234
NR = 8                   # output rows per chunk (N = NR*64 = 512 <= PSUM bank)
XLEN = 3100              # per-block x DMA length (covers max read f=3099)

MM_DT = os.environ.get("KLSTM_MM_DT", "bf16")   # "bf16" | "f32r"

X_TAPS_A = [(0, 0), (0, 1), (0, 2), (1, 0)]
X_TAPS_B = [(1, 1), (1, 2), (2, 0), (2, 1)]

_CACHE = {}


def _build_program():
    import concourse.mybir as mybir
    import concourse.tile as tile
    from concourse import bacc

    f32 = mybir.dt.float32
    dtm = mybir.dt.bfloat16 if MM_DT == "bf16" else mybir.dt.float32r

    nc = bacc.Bacc("TRN2", target_bir_lowering=False, debug=False, num_devices=8)

    xp_d = nc.dram_tensor("xp", [T, C, FLAT], dtm, kind="ExternalInput")
    wx1_d = nc.dram_tensor("wx1", [128, 256], dtm, kind="ExternalInput")
    wx2_d = nc.dram_tensor("wx2", [128, 256], dtm, kind="ExternalInput")
    wa_d = nc.dram_tensor("wa", [128, 768], dtm, kind="ExternalInput")
    wb2_d = nc.dram_tensor("wb2", [128, 256], dtm, kind="ExternalInput")
    wc_d = nc.dram_tensor("wc", [96, 256], dtm, kind="ExternalInput")
    bias_d = nc.dram_tensor("bias", [128, 2], f32, kind="ExternalInput")
    out_d = nc.dram_tensor("out", [T, HID, 32 * 64], f32, kind="ExternalOutput")

    Sigmoid = mybir.ActivationFunctionType.Sigmoid
    Tanh = mybir.ActivationFunctionType.Tanh

    with tile.TileContext(nc) as tc:
        with tc.tile_pool(name="const", bufs=1) as constp, \
             tc.tile_pool(name="xpool", bufs=3 if MM_DT == "bf16" else 2) as xpool, \
             tc.tile_pool(name="hpool", bufs=1) as hpool, \
             tc.tile_pool(name="cpool", bufs=1) as cpool, \
             tc.tile_pool(name="psum", bufs=4, space="PSUM") as psum, \
             tc.tile_pool(name="ifsp", bufs=4) as ifsp, \
             tc.tile_pool(name="smallp", bufs=4) as smallp, \
             tc.tile_pool(name="houtp", bufs=4) as houtp:

            wx1_s = constp.tile([128, 256], dtm)
            wx2_s = constp.tile([128, 256], dtm)
            wa_s = constp.tile([128, 768], dtm)
            wb2_s = constp.tile([128, 256], dtm)
            wc_s = constp.tile([96, 256], dtm)
            bias_s = constp.tile([128, 2], f32)
            for s_, d_ in [(wx1_s, wx1_d), (wx2_s, wx2_d), (wa_s, wa_d),
                           (wb2_s, wb2_d), (wc_s, wc_d), (bias_s, bias_d)]:
                nc.sync.dma_start(s_[:], d_[:])

            # ping-pong h tiles (shifted partition blocks, see module docstring)
            hA = [hpool.tile([128, FLAT], dtm, tag=f"hA{i}", name=f"hA{i}")
                  for i in range(2)]
            hB2 = [hpool.tile([128, FLAT], dtm, tag=f"hB2{i}", name=f"hB2{i}")
                   for i in range(2)]
            # cmb: parts 0-31 x tap (2,2) [per-step], parts 32-95 h tap (1,2)
            cmb = [hpool.tile([96, FLAT], dtm, tag=f"cmb{i}", name=f"cmb{i}")
                   for i in range(2)]
            for t_ in hA + hB2 + cmb:
                nc.gpsimd.memset(t_[:] if MM_DT == "bf16" else t_[:].bitcast(f32),
                                 0.0)

            c_s = cpool.tile([64, 47 * 64], f32)

            def load_x(t):
                # x tiles for step t: TA (4 blocks), TB (4 blocks), cmb x-block
                # (issued from the mostly-idle gpsimd queue, one step ahead)
                xa = xpool.tile([128, FLAT], dtm, tag="xa", name="xa")
                xb = xpool.tile([128, FLAT], dtm, tag="xb", name="xb")
                for b3, (dy, dx) in enumerate(X_TAPS_A):
                    s = dy * WP + dx
                    nc.gpsimd.dma_start(xa[32 * b3:32 * b3 + 32, 0:XLEN],
                                        xp_d[t - 1, :, s:s + XLEN])
                for b3, (dy, dx) in enumerate(X_TAPS_B):
                    s = dy * WP + dx
                    nc.gpsimd.dma_start(xb[32 * b3:32 * b3 + 32, 0:XLEN],
                                        xp_d[t - 1, :, s:s + XLEN])
                nc.gpsimd.dma_start(cmb[t % 2][0:32, 0:XLEN],
                                    xp_d[t - 1, :, 134:134 + XLEN])
                return xa, xb

            xtiles = load_x(1)
            for t in range(1, T + 1):
                R = 48 - t
                xa, xb = xtiles
                if t < T:
                    xtiles = load_x(t + 1)

                hAp, hB2p, cmbp = hA[(t - 1) % 2], hB2[(t - 1) % 2], cmb[t % 2]
                hAc, hB2c, cmbn = hA[t % 2], hB2[t % 2], cmb[(t + 1) % 2]
                xav = xa[:].rearrange("p (y x) -> p y x", x=WP)
                xbv = xb[:].rearrange("p (y x) -> p y x", x=WP)
                hAv = hAp[:].rearrange("p (y x) -> p y x", x=WP)
                hB2v = hB2p[:].rearrange("p (y x) -> p y x", x=WP)
                cmbv = cmbp[:].rearrange("p (y x) -> p y x", x=WP)

                nchunk = (R + NR - 1) // NR
                for q in range(nchunk):
                    y0 = NR * q
                    nr = min(NR, R - y0)
                    N = nr * 64
                    ps = [psum.tile([128, 512], f32, tag="psA", name="psA"),
                          psum.tile([128, 512], f32, tag="psB", name="psB")]
                    for h in range(2):
                        pt = ps[h]
                        hs = h * 128
                        nc.tensor.matmul(pt[:, :N], wx1_s[:, hs:hs + 128],
                                         xav[:, y0:y0 + nr, 0:64],
                                         start=True, stop=False)
                        nc.tensor.matmul(pt[:, :N], wx2_s[:, hs:hs + 128],
                                         xbv[:, y0:y0 + nr, 0:64],
                                         start=False, stop=False)
                        if t > 1:
                            for dy in range(3):
                                nc.tensor.matmul(
                                    pt[:, :N],
                                    wa_s[:, (dy * 2 + h) * 128:(dy * 2 + h + 1) * 128],
                                    hAv[:, y0 + dy:y0 + dy + nr, 0:64],
                                    start=False, stop=False)
                            nc.tensor.matmul(pt[:, :N], wb2_s[:, hs:hs + 128],
                                             hB2v[:, y0:y0 + nr, 0:64],
                                             start=False, stop=False)
                            nc.tensor.matmul(pt[:, :N], wc_s[:, hs:hs + 128],
                                             cmbv[0:96, y0:y0 + nr, 0:64],
                                             start=False, stop=True)
                        else:
                            nc.tensor.matmul(pt[:, :N], wc_s[0:32, hs:hs + 128],
                                             cmbv[0:32, y0:y0 + nr, 0:64],
                                             start=False, stop=True)
                    is_ = ifsp.tile([64, 512], f32, tag="is", name="is_")
                    nc.scalar.activation(is_[:, :N], ps[0][0:64, :N], Sigmoid,
                                         bias=bias_s[0:64, 0:1])
                    fs_ = ifsp.tile([64, 512], f32, tag="fs", name="fs_")
                    nc.scalar.activation(fs_[:, :N], ps[0][64:128, :N], Sigmoid,
                                         bias=bias_s[64:128, 0:1])
                    gt = smallp.tile([64, 512], f32, tag="gt")
                    nc.scalar.activation(gt[:, :N], ps[1][0:64, :N], Tanh,
                                         bias=bias_s[0:64, 1:2])
                    os_ = smallp.tile([64, 512], f32, tag="os")
                    nc.scalar.activation(os_[:, :N], ps[1][64:128, :N], Sigmoid,
                                         bias=bias_s[64:128, 1:2])
                    c_sl = c_s[:, y0 * 64:y0 * 64 + N]
                    if t == 1:
                        nc.vector.tensor_mul(c_sl, is_[:, :N], gt[:, :N])
                    else:
                        t1 = smallp.tile([64, 512], f32, tag="t1")
                        nc.vector.tensor_mul(t1[:, :N], is_[:, :N], gt[:, :N])
                        nc.vector.tensor_mul(c_sl, fs_[:, :N], c_sl)
                        nc.vector.tensor_add(c_sl, c_sl, t1[:, :N])
                    tc_ = smallp.tile([64, 512], f32, tag="tc")
                    nc.scalar.activation(tc_[:, :N], c_sl, Tanh)
                    hout = houtp.tile([64, 512], dtm, tag="hout", name="hout")
                    if t < T:
                        nc.vector.tensor_mul(hout[:, :N], os_[:, :N], tc_[:, :N])
                        h3 = hout[:, :N].rearrange("p (y x) -> p y x", x=64)
                        hAcv = hAc[:].rearrange("p (y x) -> p y x", x=WP)
                        hB2cv = hB2c[:].rearrange("p (y x) -> p y x", x=WP)
                        cmbnv = cmbn[:].rearrange("p (y x) -> p y x", x=WP)
                        # HA: shifts 0, 1
                        nc.sync.dma_start(
                            hAcv[0:64, y0 + 1:y0 + 1 + nr, 1:65], h3)
                        nc.sync.dma_start(
                            hAcv[64:128, y0 + 1:y0 + 1 + nr, 0:64], h3)
                        # HB2: shifts 2, 134
                        nc.sync.dma_start(
                            hB2cv[0:64, y0 + 1:y0 + 1 + nr, 0:63], h3[:, :, 1:64])
                        if q == 0:
                            nc.sync.dma_start(
                                hB2cv[64:128, 0:nr - 1, 0:63],
                                h3[:, 1:nr, 1:64])
                        else:
                            nc.sync.dma_start(
                                hB2cv[64:128, y0 - 1:y0 - 1 + nr, 0:63],
                                h3[:, :, 1:64])
                        # CMB h block: shift 68 (parts 32-95)
                        nc.sync.dma_start(
                            cmbnv[32:96, y0:y0 + nr, 0:63], h3[:, :, 1:64])
                    if y0 < 32:
                        if MM_DT == "bf16":
                            # separate f32 product for the output (gpsimd is idle)
                            houtf = houtp.tile([64, 512], f32, tag="houtf",
                                               name="houtf")
                            nc.gpsimd.tensor_mul(houtf[:, :512], os_[:, :512],
                                                 tc_[:, :512])
                            nc.scalar.dma_start(
                                out_d[t - 1, :, y0 * 64:y0 * 64 + 512],
                                houtf[:, :512])
                        else:
                            if t == T:
                                nc.vector.tensor_mul(hout[:, :N], os_[:, :N],
                                                     tc_[:, :N])
                            nc.scalar.dma_start(
                                out_d[t - 1, :, y0 * 64:y0 * 64 + 512],
                                hout[:, :512].bitcast(f32))
    nc.compile()
    return nc


def _host_prep(x, w_x2h, b_x2h, w_h2h, b_h2h):
    """Build the 8 per-core input maps."""
    import ml_dtypes
    np_dtm = ml_dtypes.bfloat16 if MM_DT == "bf16" else np.float32

    x = np.ascontiguousarray(np.asarray(x, np.float32))
    w_x2h = np.asarray(w_x2h, np.float32)
    b_x2h = np.asarray(b_x2h, np.float32)
    w_h2h = np.asarray(w_h2h, np.float32)
    b_h2h = np.asarray(b_h2h, np.float32)

    bias = np.zeros((128, 2), np.float32)
    bsum = b_x2h + b_h2h
    bias[:, 0] = bsum[0:128]
    bias[:, 1] = bsum[128:256]

    in_maps = []
    packed_w = {}
    for parity in range(2):
        wx_f = w_x2h if parity == 0 else w_x2h[:, :, ::-1, :]
        wh_f = w_h2h if parity == 0 else w_h2h[:, :, ::-1, :]
        wx1 = np.zeros((128, 2, 128), np.float32)
        wx2 = np.zeros((128, 2, 128), np.float32)
        wa = np.zeros((128, 3, 2, 128), np.float32)
        wb2 = np.zeros((128, 2, 128), np.float32)
        wc = np.zeros((96, 2, 128), np.float32)
        for hh in range(2):
            oc = slice(hh * 128, (hh + 1) * 128)
            for b3, (dy, dx) in enumerate(X_TAPS_A):
                wx1[32 * b3:32 * b3 + 32, hh, :] = wx_f[oc, :, dy, dx].T
            for b3, (dy, dx) in enumerate(X_TAPS_B):
                wx2[32 * b3:32 * b3 + 32, hh, :] = wx_f[oc, :, dy, dx].T
            for dy in range(3):
                for b3 in range(2):
                    wa[64 * b3:64 * b3 + 64, dy, hh, :] = wh_f[oc, :, dy, b3].T
            wb2[0:64, hh, :] = wh_f[oc, :, 0, 2].T
            wb2[64:128, hh, :] = wh_f[oc, :, 2, 2].T
            wc[0:32, hh, :] = wx_f[oc, :, 2, 2].T
            wc[32:96, hh, :] = wh_f[oc, :, 1, 2].T
        packed_w[parity] = tuple(
            np.ascontiguousarray(a.reshape(a.shape[0], -1).astype(np_dtm))
            for a in (wx1, wx2, wa, wb2, wc))

    for core in range(8):
        b, parity = core // 2, core % 2
        xv = x[:, b]
        if parity == 1:
            xv = xv[:, :, ::-1, :]
        xp = np.zeros((T, C, HP, WP), np.float32)
        xp[:, :, 1:49, 1:65] = xv[:, :, 0:48, :]
        wx1, wx2, wa, wb2, wc = packed_w[parity]
        in_maps.append({
            "xp": np.ascontiguousarray(xp.reshape(T, C, FLAT).astype(np_dtm)),
            "wx1": wx1, "wx2": wx2, "wa": wa, "wb2": wb2, "wc": wc,
            "bias": bias,
        })
    return in_maps


def kernel(x, w_x2h, b_x2h, w_h2h, b_h2h, _trace=False, _tmpdir=None):
    from concourse.bass_utils import run_bass_kernel_spmd

    if "nc" not in _CACHE:
        _CACHE["nc"] = _build_program()
    nc = _CACHE["nc"]

    in_maps = _host_prep(x, w_x2h, b_x2h, w_h2h, b_h2h)
    kw = {}
    if _trace:
        kw = dict(trace=True, tmpdir=_tmpdir)
    res = run_bass_kernel_spmd(nc, in_maps, core_ids=list(range(8)), **kw)

    full = np.zeros((T, B, HID, H, W), np.float32)
    for core in range(8):
        b, parity = core // 2, core % 2
        out = res.results[core]["out"].reshape(T, HID, 32, 64)
        if parity == 0:
            full[:, b, :, 0:32] = out
        else:
            full[:, b, :, 32:64] = out[:, :, ::-1, :]
    if _trace:
        return full, res
    return full

